# revision 6
# baseline (speedup 1.0000x reference)
"""Trainium2 Bass kernel for nn_Decoder_45921790329638 (GRU decoder + sampling).

Self-contained: takes FULL inputs, shards across 8 NeuronCores internally
(vocab-TP for out_w/logits, gate-TP for the GRU), returns FULL outputs.

  - out_w sharded by vocab: core c holds columns [c*6283,(c+1)*6283) of the
    (padded-to-50264) vocab, stored transposed as K-tiles resident in SBUF.
  - GRU is gate-parallel: core c computes an 88-wide slice of r/z/n for all
    256 batch rows; h slices are all-gathered (transposed) each step.
  - jax.random.categorical(k, logits) == argmax(logits + gumbel(k)); key 42
    is fixed, so the Gumbel noise is a constant: precomputed on host CPU
    (bit-exact jax threefry) and streamed per step. Each core adds noise to
    its logits shard, takes per-chunk max8/max_index (DVE), combines with a
    min-index tie-break, and the 8 cores exchange (max, argmax) via a tiny
    AllGather. The winner drives an indirect-DMA embedding gather.
"""
import numpy as np

# ---- problem constants ----
VOCAB, ATTR, B = 50257, 16, 256
EMB, ATTR_EMB, HID, ZDIM = 300, 200, 700, 500
MAXLEN = 30
SOS_IDX, EOS_IDX = 1, 2
NCORES = 8
VSH = 6283            # vocab shard per core (8*6283 = 50264 >= 50257)
VPAD = VSH * NCORES
HSL = 88              # hidden slice per core (8*88 = 704)
HPAD = HSL * NCORES
GSL = 3 * HSL         # 264 gate columns per core
KT = 6                # k-tiles for HID (704 -> 6x128, last 64 rows)
KTE = 3               # k-tiles for EMB (300)
NH = 2                # batch halves
CHUNK = 1024
NCH = (VSH + CHUNK - 1) // CHUNK   # 7
BIG = float(2 ** 23)

MODE = "f32"   # logits matmul: "f32" | "f32r"

_cache = {}


def _gumbel_noise():
    import jax
    import jax.numpy as jnp
    cpu = jax.devices("cpu")[0]
    with jax.default_device(cpu):
        keys = jax.random.split(jax.random.key(42), MAXLEN)
        f = jax.jit(lambda k: jax.random.gumbel(k, (B, VOCAB), jnp.float32),
                    backend="cpu")
        outs = [np.asarray(f(k)) for k in keys]
    g = np.full((MAXLEN, B, VPAD), -1e30, np.float32)
    g[:, :, :VOCAB] = np.stack(outs)
    return g


def _build_bass():
    import concourse.bass as bass
    import concourse.mybir as mybir
    import concourse.tile as tile
    from concourse import bacc

    f32 = mybir.dt.float32
    f32r = mybir.dt.float32r
    i32 = mybir.dt.int32
    u32 = mybir.dt.uint32
    AF = mybir.ActivationFunctionType
    OP = mybir.AluOpType
    AX = mybir.AxisListType

    nc = bacc.Bacc("TRN2", target_bir_lowering=False, debug=False,
                   num_devices=NCORES)

    d_wkn = nc.dram_tensor("wkn", [KT, 128, VSH], f32, kind="ExternalInput").ap()
    d_noise = nc.dram_tensor("noise", [MAXLEN, NH, 128, VSH], f32,
                             kind="ExternalInput").ap()
    d_emb = nc.dram_tensor("embw", [VOCAB, EMB], f32, kind="ExternalInput").ap()
    d_wih = nc.dram_tensor("wih", [KTE, 128, GSL], f32, kind="ExternalInput").ap()
    d_whh = nc.dram_tensor("whh", [KT, 128, GSL], f32, kind="ExternalInput").ap()
    d_bih = nc.dram_tensor("bih", [128, GSL], f32, kind="ExternalInput").ap()
    d_bhh = nc.dram_tensor("bhh", [128, GSL], f32, kind="ExternalInput").ap()
    d_h0t = nc.dram_tensor("h0t", [KT, 128, B], f32, kind="ExternalInput").ap()
    d_h0sl = nc.dram_tensor("h0sl", [128, NH, HSL], f32, kind="ExternalInput").ap()
    d_cbase = nc.dram_tensor("cbase", [128, NH, NCH], f32,
                             kind="ExternalInput").ap()
    d_ident = nc.dram_tensor("ident", [128, 128], f32, kind="ExternalInput").ap()

    d_hy = nc.dram_tensor("hy_part", [MAXLEN + 1, NH, 128, HSL], f32,
                          kind="ExternalOutput").ap()
    d_y = nc.dram_tensor("y_out", [128, NH, 32], i32, kind="ExternalOutput").ap()

    with tile.TileContext(nc) as tc:
        import contextlib
        with contextlib.ExitStack() as ctx:
            sb1 = ctx.enter_context(tc.tile_pool(name="res", bufs=1))
            sbn = ctx.enter_context(tc.tile_pool(name="noise", bufs=4))
            ps_log = ctx.enter_context(
                tc.tile_pool(name="pslog", bufs=2, space="PSUM"))
            ps_gh = ctx.enter_context(
                tc.tile_pool(name="psgh", bufs=2, space="PSUM"))
            ps_sh = ctx.enter_context(
                tc.tile_pool(name="pssh", bufs=2, space="PSUM"))
            dram = ctx.enter_context(
                tc.tile_pool(name="dx", bufs=3, space="DRAM"))

            lw_dt = {"f32": f32, "f32r": f32r}[MODE]
            w_r = sb1.tile([128, KT, VSH], lw_dt, tag="w_r")
            wih_t = sb1.tile([128, KTE, GSL], f32, tag="wih")
            whh_t = sb1.tile([128, KT, GSL], f32, tag="whh")
            bih_t = sb1.tile([128, GSL], f32, tag="bih")
            bhh_t = sb1.tile([128, GSL], f32, tag="bhh")
            cbase_t = sb1.tile([128, NH, NCH], f32, tag="cbase")
            ident_t = sb1.tile([128, 128], f32, tag="ident")
            hT = sb1.tile([128, KT, B], f32, tag="hT")
            if MODE == "f32r":
                hT_r = sb1.tile([128, KT, B], lw_dt, tag="hT_r", name="hT_r")
            else:
                hT_r = None
            y_sb = sb1.tile([128, NH, 32], i32, tag="y")
            tok_i32 = sb1.tile([128, NH, 1], i32, tag="tok")
            m8_all = sb1.tile([128, NH, NCH, 8], f32, tag="m8")
            i8_all = sb1.tile([128, NH, NCH, 8], u32, tag="i8")
            gh_sb = sb1.tile([128, NH, GSL], f32, tag="gh_sb")
            gi_sb = sb1.tile([128, NH, GSL], f32, tag="gi_sb")
            hsl = [sb1.tile([128, NH, HSL], f32, tag=f"hsl{i}",
                            name=f"hsl{i}") for i in range(2)]
            hslT_send = sb1.tile([HSL, B], f32, tag="hslT")
            x_sb = sb1.tile([128, NH, EMB], f32, tag="x_sb")
            xT = sb1.tile([128, KTE, B], f32, tag="xT")
            ex_in = sb1.tile([128, 4], f32, tag="ex_in")
            exl = sb1.tile([128, NCORES, 4], f32, tag="exl")
            A_sb = sb1.tile([128, NH, 2 * HSL], f32, tag="A_sb")
            n_sb = sb1.tile([128, NH, HSL], f32, tag="n_sb")
            t1_sb = sb1.tile([128, NH, HSL], f32, tag="t1_sb")
            Mloc = sb1.tile([128, NH, 1], f32, tag="Mloc")
            if32 = sb1.tile([128, NH, NCH], f32, tag="if32")
            eq_sb = sb1.tile([128, NH, NCH], f32, tag="eq")
            MG = sb1.tile([128, NH, 1], f32, tag="MG")
            eq2 = sb1.tile([128, NH, NCORES], f32, tag="eq2")
            av_t = sb1.tile([128, NH, NCORES], f32, tag="avt")
            Ag = sb1.tile([128, NH, 1], f32, tag="Ag")

            # ---------- init ----------
            nc.sync.dma_start(wih_t[:], d_wih[:].rearrange("k p g -> p k g"))
            nc.sync.dma_start(whh_t[:], d_whh[:].rearrange("k p g -> p k g"))
            nc.sync.dma_start(bih_t[:], d_bih[:])
            nc.sync.dma_start(bhh_t[:], d_bhh[:])
            nc.sync.dma_start(cbase_t[:], d_cbase[:])
            nc.sync.dma_start(ident_t[:], d_ident[:])
            nc.sync.dma_start(hT[:], d_h0t[:].rearrange("k p b -> p k b"))
            nc.sync.dma_start(hsl[0][:], d_h0sl[:])
            nc.vector.memset(xT[:], 0.0)
            nc.vector.memset(tok_i32[:], SOS_IDX)
            nc.vector.memset(y_sb[:], 0)

            for k in range(KT):
                for c0 in range(0, VSH, CHUNK):
                    cw = min(CHUNK, VSH - c0)
                    stg = sbn.tile([128, CHUNK], f32, tag="big")
                    nc.sync.dma_start(stg[:, :cw], d_wkn[k, :, c0:c0 + cw])
                    nc.vector.tensor_copy(w_r[:, k, c0:c0 + cw], stg[:, :cw])

            if MODE == "f32r":
                nc.vector.tensor_copy(hT_r[:], hT[:])

            def lhs(k, half):
                src = hT_r if MODE == "f32r" else hT
                return src[:, k, half * 128:(half + 1) * 128]

            def emit_gh():
                for half in range(NH):
                    pg = ps_gh.tile([128, GSL], f32, tag="ghp")
                    for k in range(KT):
                        nc.tensor.matmul(
                            pg[:], hT[:, k, half * 128:(half + 1) * 128],
                            whh_t[:, k, :],
                            start=(k == 0), stop=(k == KT - 1))
                    nc.vector.tensor_add(gh_sb[:, half, :], pg[:], bhh_t[:])

            emit_gh()

            for t in range(MAXLEN + 1):
                cur, nxt = hsl[t % 2], hsl[(t + 1) % 2]
                for half in range(NH):
                    nc.gpsimd.indirect_dma_start(
                        out=x_sb[:, half, :], out_offset=None,
                        in_=d_emb[:],
                        in_offset=bass.IndirectOffsetOnAxis(
                            ap=tok_i32[:, half, :], axis=0))
                for half in range(NH):
                    for k in range(KTE):
                        cw = min(128, EMB - k * 128)
                        pt = ps_sh.tile([128, 128], f32, tag="shp")
                        nc.tensor.transpose(
                            pt[:cw, :], x_sb[:, half, k * 128:k * 128 + cw],
                            ident_t[:])
                        nc.scalar.copy(xT[:cw, k, half * 128:(half + 1) * 128],
                                       pt[:cw, :])
                for half in range(NH):
                    pgi = ps_sh.tile([128, GSL], f32, tag="shp")
                    for k in range(KTE):
                        nc.tensor.matmul(
                            pgi[:], xT[:, k, half * 128:(half + 1) * 128],
                            wih_t[:, k, :],
                            start=(k == 0), stop=(k == KTE - 1))
                    nc.vector.tensor_add(gi_sb[:, half, :], pgi[:], bih_t[:])
                # gates
                nc.vector.tensor_add(A_sb[:], gi_sb[:, :, 0:2 * HSL],
                                     gh_sb[:, :, 0:2 * HSL])
                nc.scalar.activation(A_sb[:], A_sb[:], AF.Sigmoid)
                nc.vector.tensor_mul(t1_sb[:], A_sb[:, :, 0:HSL],
                                     gh_sb[:, :, 2 * HSL:3 * HSL])
                nc.vector.tensor_add(n_sb[:], gi_sb[:, :, 2 * HSL:3 * HSL],
                                     t1_sb[:])
                nc.scalar.activation(n_sb[:], n_sb[:], AF.Tanh)
                nc.vector.tensor_sub(t1_sb[:], cur[:], n_sb[:])
                nc.vector.tensor_mul(t1_sb[:], A_sb[:, :, HSL:2 * HSL],
                                     t1_sb[:])
                nc.vector.tensor_add(nxt[:], n_sb[:], t1_sb[:])
                nc.sync.dma_start(d_hy[t].rearrange("h p u -> p h u"), nxt[:])
                if t == MAXLEN:
                    break
                # exchange h slices (transposed)
                for half in range(NH):
                    pt = ps_sh.tile([128, 128], f32, tag="shp")
                    nc.tensor.transpose(pt[:HSL, :], nxt[:, half, :], ident_t[:])
                    nc.scalar.copy(hslT_send[:, half * 128:(half + 1) * 128],
                                   pt[:HSL, :])
                hx_in = dram.tile([HSL, B], f32, tag="hx_in")
                hx_out = dram.tile([NCORES, HSL, B], f32, tag="hx_out")
                nc.sync.dma_start(hx_in[:], hslT_send[:])
                nc.gpsimd.collective_compute(
                    "AllGather", OP.bypass,
                    replica_groups=[list(range(NCORES))],
                    ins=[hx_in.opt()], outs=[hx_out.opt()])
                hx_flat = hx_out[:].rearrange("c r b -> (c r) b")
                for k in range(KT):
                    rw = min(128, HPAD - k * 128)
                    nc.sync.dma_start(hT[:rw, k, :],
                                      hx_flat[k * 128:k * 128 + rw, :])
                if MODE == "f32r":
                    nc.vector.tensor_copy(hT_r[:], hT[:])

                # logits + sampling
                for half in range(NH):
                    for c in range(NCH):
                        c0 = c * CHUNK
                        cw = min(CHUNK, VSH - c0)
                        pl = ps_log.tile([128, CHUNK], f32, tag="pl")
                        for s0 in range(0, cw, 512):
                            sw = min(512, cw - s0)
                            for k in range(KT):
                                nc.tensor.matmul(
                                    pl[:, s0:s0 + sw], lhs(k, half),
                                    w_r[:, k, c0 + s0:c0 + s0 + sw],
                                    start=(k == 0), stop=(k == KT - 1))
                        nz = sbn.tile([128, CHUNK], f32, tag="big")
                        nc.sync.dma_start(nz[:, :cw],
                                          d_noise[t, half, :, c0:c0 + cw])
                        nc.vector.tensor_add(nz[:, :cw], pl[:, :cw], nz[:, :cw])
                        nc.vector.max(out=m8_all[:, half, c], in_=nz[:, :cw])
                        nc.vector.max_index(out=i8_all[:, half, c],
                                            in_max=m8_all[:, half, c],
                                            in_values=nz[:, :cw])
                emit_gh()
                # local combine (both halves at once)
                m_cat = m8_all[:, :, :, 0]
                nc.vector.tensor_reduce(Mloc[:].squeeze(2), m_cat,
                                        axis=AX.X, op=OP.max)
                nc.vector.tensor_copy(if32[:], i8_all[:, :, :, 0])
                nc.vector.tensor_add(if32[:], if32[:], cbase_t[:])
                nc.vector.tensor_tensor(
                    out=eq_sb[:], in0=m_cat,
                    in1=Mloc[:].to_broadcast([128, NH, NCH]), op=OP.is_equal)
                nc.vector.tensor_scalar_sub(if32[:], if32[:], BIG)
                nc.vector.tensor_mul(if32[:], eq_sb[:], if32[:])
                nc.vector.tensor_scalar_add(if32[:], if32[:], BIG)
                nc.vector.tensor_reduce(Ag[:].squeeze(2), if32[:],
                                        axis=AX.X, op=OP.min)
                nc.vector.tensor_copy(ex_in[:, 0:2], Mloc[:].squeeze(2))
                nc.vector.tensor_copy(ex_in[:, 2:4], Ag[:].squeeze(2))
                ei = dram.tile([128, 4], f32, tag="ex_i")
                eo = dram.tile([NCORES, 128, 4], f32, tag="ex_o")
                nc.sync.dma_start(ei[:], ex_in[:])
                nc.gpsimd.collective_compute(
                    "AllGather", OP.bypass,
                    replica_groups=[list(range(NCORES))],
                    ins=[ei.opt()], outs=[eo.opt()])
                nc.sync.dma_start(exl[:], eo[:].rearrange("c p v -> p c v"))
                m_v = exl[:, :, 0:2].rearrange("p c v -> p v c")
                a_v = exl[:, :, 2:4].rearrange("p c v -> p v c")
                nc.vector.tensor_reduce(MG[:].squeeze(2), m_v, axis=AX.X,
                                        op=OP.max)
                nc.vector.tensor_tensor(
                    out=eq2[:], in0=m_v,
                    in1=MG[:].to_broadcast([128, NH, NCORES]), op=OP.is_equal)
                nc.vector.tensor_scalar_sub(av_t[:], a_v, BIG)
                nc.vector.tensor_mul(av_t[:], eq2[:], av_t[:])
                nc.vector.tensor_scalar_add(av_t[:], av_t[:], BIG)
                nc.vector.tensor_reduce(Ag[:].squeeze(2), av_t[:], axis=AX.X,
                                        op=OP.min)
                nc.vector.tensor_copy(tok_i32[:].squeeze(2), Ag[:].squeeze(2))
                nc.vector.tensor_copy(y_sb[:, :, t], tok_i32[:].squeeze(2))
                if t == MAXLEN - 1:
                    nc.vector.memset(tok_i32[:], EOS_IDX)

            nc.sync.dma_start(d_y[:], y_sb[:])

    nc.compile()
    return nc


def _host_prep(inputs):
    z = np.asarray(inputs["z"], np.float32)
    l = np.asarray(inputs["l"], np.int32)
    emb_w = np.ascontiguousarray(np.asarray(inputs["emb_w"], np.float32))
    attr_w = np.asarray(inputs["attr_w"], np.float32)
    W_ih = np.asarray(inputs["W_ih"], np.float32)
    W_hh = np.asarray(inputs["W_hh"], np.float32)
    b_ih = np.asarray(inputs["b_ih"], np.float32)
    b_hh = np.asarray(inputs["b_hh"], np.float32)
    out_w = np.asarray(inputs["out_w"], np.float32)
    out_b = np.asarray(inputs["out_b"], np.float32)

    if "noise" not in _cache:
        _cache["noise"] = _gumbel_noise()
    g = _cache["noise"] + np.pad(
        out_b, (0, VPAD - VOCAB), constant_values=0.0)[None, None, :]
    g[:, :, VOCAB:] = -1e30

    h0 = np.concatenate([z, attr_w[l]], axis=1)
    h0p = np.zeros((B, HPAD), np.float32)
    h0p[:, :HID] = h0
    h0T = h0p.T
    h0t = np.zeros((KT, 128, B), np.float32)
    for k in range(KT):
        r = min(128, HPAD - k * 128)
        h0t[k, :r] = h0T[k * 128:k * 128 + r]

    wT = np.zeros((KT * 128, VPAD), np.float32)
    wT[:HID, :VOCAB] = out_w.T
    ident = np.eye(128, dtype=np.float32)

    in_maps = []
    for c in range(NCORES):
        base = c * VSH
        wkn = np.ascontiguousarray(
            wT[:, base:base + VSH].reshape(KT, 128, VSH))
        noise_c = np.ascontiguousarray(
            g[:, :, base:base + VSH].reshape(MAXLEN, NH, 128, VSH))
        u0 = c * HSL
        rows = np.array([gate * HID + u0 + u if u0 + u < HID else -1
                         for gate in range(3) for u in range(HSL)])
        ok = rows >= 0
        wih_sl = np.zeros((GSL, EMB), np.float32)
        whh_sl = np.zeros((GSL, HPAD), np.float32)
        b_ih_sl = np.zeros((GSL,), np.float32)
        b_hh_sl = np.zeros((GSL,), np.float32)
        wih_sl[ok] = W_ih[rows[ok]]
        whh_sl[ok, :HID] = W_hh[rows[ok]]
        b_ih_sl[ok] = b_ih[rows[ok]]
        b_hh_sl[ok] = b_hh[rows[ok]]
        wih_t = np.zeros((KTE, 128, GSL), np.float32)
        wihT = wih_sl.T
        for k in range(KTE):
            r = min(128, EMB - k * 128)
            wih_t[k, :r] = wihT[k * 128:k * 128 + r]
        whh_t = np.zeros((KT, 128, GSL), np.float32)
        whhT = whh_sl.T  # (704, 264)
        for k in range(KT):
            r = min(128, HPAD - k * 128)
            whh_t[k, :r] = whhT[k * 128:k * 128 + r]
        h0sl = np.zeros((128, NH, HSL), np.float32)
        for half in range(NH):
            h0sl[:, half, :] = h0p[half * 128:(half + 1) * 128, u0:u0 + HSL]
        cbase = np.zeros((128, NH, NCH), np.float32)
        for ci in range(NCH):
            cbase[:, :, ci] = base + ci * CHUNK
        in_maps.append({
            "wkn": wkn, "noise": noise_c, "embw": emb_w,
            "wih": wih_t, "whh": whh_t,
            "bih": np.broadcast_to(b_ih_sl, (128, GSL)).copy(),
            "bhh": np.broadcast_to(b_hh_sl, (128, GSL)).copy(),
            "h0t": h0t, "h0sl": h0sl, "cbase": cbase, "ident": ident,
        })
    return in_maps


def kernel(**inputs):
    from concourse import bass_utils
    if "nc" not in _cache:
        _cache["nc"] = _build_bass()
    nc = _cache["nc"]
    in_maps = _host_prep(inputs)
    res = bass_utils.run_bass_kernel_spmd(
        nc, in_maps, core_ids=list(range(NCORES)),
        trace=bool(_cache.get("trace", False)))
    _cache["last_result"] = res
    outs = res.results
    hy = np.zeros((B, MAXLEN + 1, HID), np.float32)
    for c in range(NCORES):
        hp = outs[c]["hy_part"]  # (31, 2, 128, 88)
        u0 = c * HSL
        w = min(HSL, HID - u0)
        if w <= 0:
            continue
        for half in range(NH):
            hy[half * 128:(half + 1) * 128, :, u0:u0 + w] = \
                hp[:, half, :, :w].transpose(1, 0, 2)
    yo = outs[0]["y_out"]
    y = np.zeros((B, MAXLEN + 1), np.int32)
    for half in range(NH):
        y[half * 128:(half + 1) * 128, :MAXLEN] = yo[:, half, :MAXLEN]
    y[:, MAXLEN] = EOS_IDX
    return hy, y


# revision 8
# speedup vs baseline: 1.9338x; 1.9338x over previous
"""Trainium2 Bass kernel for nn_Decoder_45921790329638 (GRU decoder + sampling).

Self-contained: takes FULL inputs, shards across 8 NeuronCores internally
(vocab-TP for out_w/logits, gate-TP for the GRU), returns FULL outputs.

  - out_w sharded by vocab: core c holds columns [c*6283,(c+1)*6283) of the
    (padded-to-50264) vocab, stored transposed as K-tiles resident in SBUF.
  - GRU is gate-parallel: core c computes an 88-wide slice of r/z/n for all
    256 batch rows; h slices are all-gathered (transposed) each step.
  - jax.random.categorical(k, logits) == argmax(logits + gumbel(k)); key 42
    is fixed, so the Gumbel noise is a constant: precomputed on host CPU
    (bit-exact jax threefry) and streamed per step. Each core adds noise to
    its logits shard, takes per-chunk max8/max_index (DVE), combines with a
    min-index tie-break, and the 8 cores exchange (max, argmax) via a tiny
    AllGather. The winner drives an indirect-DMA embedding gather.
"""
import numpy as np

# ---- problem constants ----
VOCAB, ATTR, B = 50257, 16, 256
EMB, ATTR_EMB, HID, ZDIM = 300, 200, 700, 500
MAXLEN = 30
SOS_IDX, EOS_IDX = 1, 2
NCORES = 8
VSH = 6284            # vocab shard per core (8*6284 = 50272 >= 50257)
VPAD = VSH * NCORES
HSL = 88              # hidden slice per core (8*88 = 704)
HPAD = HSL * NCORES
GSL = 3 * HSL         # 264 gate columns per core
KT = 6                # k-tiles for HID (704 -> 6x128, last 64 rows)
KTE = 3               # k-tiles for EMB (300)
NH = 2                # batch halves
CHUNK = 1024
NCH = (VSH + CHUNK - 1) // CHUNK   # 7
BIG = float(2 ** 23)

MODE = "f32r"   # logits matmul: "f32" | "f32r"

_cache = {}


def _gumbel_noise():
    import jax
    import jax.numpy as jnp
    cpu = jax.devices("cpu")[0]
    with jax.default_device(cpu):
        keys = jax.random.split(jax.random.key(42), MAXLEN)
        f = jax.jit(lambda k: jax.random.gumbel(k, (B, VOCAB), jnp.float32),
                    backend="cpu")
        outs = [np.asarray(f(k)) for k in keys]
    g = np.full((MAXLEN, B, VPAD), -1e30, np.float32)
    g[:, :, :VOCAB] = np.stack(outs)
    return g


def _build_bass():
    import concourse.bass as bass
    import concourse.mybir as mybir
    import concourse.tile as tile
    from concourse import bacc

    f32 = mybir.dt.float32
    f32r = mybir.dt.float32r
    i32 = mybir.dt.int32
    u32 = mybir.dt.uint32
    AF = mybir.ActivationFunctionType
    OP = mybir.AluOpType
    AX = mybir.AxisListType

    nc = bacc.Bacc("TRN2", target_bir_lowering=False, debug=False,
                   num_devices=NCORES)

    d_wkn = nc.dram_tensor("wkn", [KT, 128, VSH], f32, kind="ExternalInput").ap()
    d_noise = nc.dram_tensor("noise", [MAXLEN, NH, 128, VSH], f32,
                             kind="ExternalInput").ap()
    d_emb = nc.dram_tensor("embw", [VOCAB, EMB], f32, kind="ExternalInput").ap()
    d_wih = nc.dram_tensor("wih", [KTE, 128, GSL], f32, kind="ExternalInput").ap()
    d_whh = nc.dram_tensor("whh", [KT, 128, GSL], f32, kind="ExternalInput").ap()
    d_bih = nc.dram_tensor("bih", [128, GSL], f32, kind="ExternalInput").ap()
    d_bhh = nc.dram_tensor("bhh", [128, GSL], f32, kind="ExternalInput").ap()
    d_h0t = nc.dram_tensor("h0t", [KT, 128, B], f32, kind="ExternalInput").ap()
    d_h0sl = nc.dram_tensor("h0sl", [128, NH, HSL], f32, kind="ExternalInput").ap()
    d_cbase = nc.dram_tensor("cbase", [128, NH, NCH], f32,
                             kind="ExternalInput").ap()
    d_ident = nc.dram_tensor("ident", [128, 128], f32, kind="ExternalInput").ap()

    d_hy = nc.dram_tensor("hy_part", [MAXLEN + 1, NH, 128, HSL], f32,
                          kind="ExternalOutput").ap()
    d_y = nc.dram_tensor("y_out", [128, NH, 32], i32, kind="ExternalOutput").ap()

    with tile.TileContext(nc) as tc:
        import contextlib
        with contextlib.ExitStack() as ctx:
            sb1 = ctx.enter_context(tc.tile_pool(name="res", bufs=1))
            sbn = ctx.enter_context(tc.tile_pool(name="noise", bufs=4))
            ps_log = ctx.enter_context(
                tc.tile_pool(name="pslog", bufs=2, space="PSUM"))
            ps_gh = ctx.enter_context(
                tc.tile_pool(name="psgh", bufs=2, space="PSUM"))
            ps_sh = ctx.enter_context(
                tc.tile_pool(name="pssh", bufs=2, space="PSUM"))
            dram = ctx.enter_context(
                tc.tile_pool(name="dx", bufs=3, space="DRAM"))

            lw_dt = {"f32": f32, "f32r": f32r}[MODE]
            w_r = sb1.tile([128, KT, VSH], lw_dt, tag="w_r")
            wih_t = sb1.tile([128, KTE, GSL], f32, tag="wih")
            whh_t = sb1.tile([128, KT, GSL], f32, tag="whh")
            bih_t = sb1.tile([128, GSL], f32, tag="bih")
            bhh_t = sb1.tile([128, GSL], f32, tag="bhh")
            cbase_t = sb1.tile([128, NH, NCH], f32, tag="cbase")
            ident_t = sb1.tile([128, 128], f32, tag="ident")
            hT = sb1.tile([128, KT, B], f32, tag="hT")
            if MODE == "f32r":
                hT_r = sb1.tile([128, KT, B], lw_dt, tag="hT_r", name="hT_r")
            else:
                hT_r = None
            y_sb = sb1.tile([128, NH, 32], i32, tag="y")
            tok_i32 = sb1.tile([128, NH, 1], i32, tag="tok")
            m8_all = sb1.tile([128, NH, NCH, 8], f32, tag="m8")
            i8_all = sb1.tile([128, NH, NCH, 8], u32, tag="i8")
            gh_sb = sb1.tile([128, NH, GSL], f32, tag="gh_sb")
            gi_sb = sb1.tile([128, NH, GSL], f32, tag="gi_sb")
            hsl = [sb1.tile([128, NH, HSL], f32, tag=f"hsl{i}",
                            name=f"hsl{i}") for i in range(2)]
            hslT_send = sb1.tile([HSL, B], f32, tag="hslT")
            x_sb = sb1.tile([128, NH, EMB], f32, tag="x_sb")
            xT = sb1.tile([128, KTE, B], f32, tag="xT")
            ex_in = sb1.tile([128, 4], f32, tag="ex_in")
            exl = sb1.tile([128, NCORES, 4], f32, tag="exl")
            A_sb = sb1.tile([128, NH, 2 * HSL], f32, tag="A_sb")
            n_sb = sb1.tile([128, NH, HSL], f32, tag="n_sb")
            t1_sb = sb1.tile([128, NH, HSL], f32, tag="t1_sb")
            Mloc = sb1.tile([128, NH, 1], f32, tag="Mloc")
            if32 = sb1.tile([128, NH, NCH], f32, tag="if32")
            eq_sb = sb1.tile([128, NH, NCH], f32, tag="eq")
            MG = sb1.tile([128, NH, 1], f32, tag="MG")
            eq2 = sb1.tile([128, NH, NCORES], f32, tag="eq2")
            av_t = sb1.tile([128, NH, NCORES], f32, tag="avt")
            Ag = sb1.tile([128, NH, 1], f32, tag="Ag")

            # ---------- init ----------
            nc.sync.dma_start(wih_t[:], d_wih[:].rearrange("k p g -> p k g"))
            nc.sync.dma_start(whh_t[:], d_whh[:].rearrange("k p g -> p k g"))
            nc.sync.dma_start(bih_t[:], d_bih[:])
            nc.sync.dma_start(bhh_t[:], d_bhh[:])
            nc.sync.dma_start(cbase_t[:], d_cbase[:])
            nc.sync.dma_start(ident_t[:], d_ident[:])
            nc.sync.dma_start(hT[:], d_h0t[:].rearrange("k p b -> p k b"))
            nc.sync.dma_start(hsl[0][:], d_h0sl[:])
            nc.vector.memset(xT[:], 0.0)
            nc.vector.memset(tok_i32[:], SOS_IDX)
            nc.vector.memset(y_sb[:], 0)

            for k in range(KT):
                for c0 in range(0, VSH, CHUNK):
                    cw = min(CHUNK, VSH - c0)
                    stg = sbn.tile([128, CHUNK], f32, tag="big")
                    nc.sync.dma_start(stg[:, :cw], d_wkn[k, :, c0:c0 + cw])
                    nc.vector.tensor_copy(w_r[:, k, c0:c0 + cw], stg[:, :cw])

            if MODE == "f32r":
                nc.vector.tensor_copy(hT_r[:], hT[:])

            def lhs(k, half):
                src = hT_r if MODE == "f32r" else hT
                return src[:, k, half * 128:(half + 1) * 128]

            def emit_gh():
                for half in range(NH):
                    pg = ps_gh.tile([128, GSL], f32, tag="ghp")
                    for k in range(KT):
                        nc.tensor.matmul(
                            pg[:], hT[:, k, half * 128:(half + 1) * 128],
                            whh_t[:, k, :],
                            start=(k == 0), stop=(k == KT - 1))
                    nc.vector.tensor_add(gh_sb[:, half, :], pg[:], bhh_t[:])

            emit_gh()

            for t in range(MAXLEN + 1):
                cur, nxt = hsl[t % 2], hsl[(t + 1) % 2]
                for half in range(NH):
                    nc.gpsimd.indirect_dma_start(
                        out=x_sb[:, half, :], out_offset=None,
                        in_=d_emb[:],
                        in_offset=bass.IndirectOffsetOnAxis(
                            ap=tok_i32[:, half, :], axis=0))
                for half in range(NH):
                    for k in range(KTE):
                        cw = min(128, EMB - k * 128)
                        pt = ps_sh.tile([128, 128], f32, tag="shp")
                        nc.tensor.transpose(
                            pt[:cw, :], x_sb[:, half, k * 128:k * 128 + cw],
                            ident_t[:])
                        nc.scalar.copy(xT[:cw, k, half * 128:(half + 1) * 128],
                                       pt[:cw, :])
                for half in range(NH):
                    pgi = ps_sh.tile([128, GSL], f32, tag="shp")
                    for k in range(KTE):
                        nc.tensor.matmul(
                            pgi[:], xT[:, k, half * 128:(half + 1) * 128],
                            wih_t[:, k, :],
                            start=(k == 0), stop=(k == KTE - 1))
                    nc.vector.tensor_add(gi_sb[:, half, :], pgi[:], bih_t[:])
                # gates
                nc.vector.tensor_add(A_sb[:], gi_sb[:, :, 0:2 * HSL],
                                     gh_sb[:, :, 0:2 * HSL])
                nc.scalar.activation(A_sb[:], A_sb[:], AF.Sigmoid)
                nc.vector.tensor_mul(t1_sb[:], A_sb[:, :, 0:HSL],
                                     gh_sb[:, :, 2 * HSL:3 * HSL])
                nc.vector.tensor_add(n_sb[:], gi_sb[:, :, 2 * HSL:3 * HSL],
                                     t1_sb[:])
                nc.scalar.activation(n_sb[:], n_sb[:], AF.Tanh)
                nc.vector.tensor_sub(t1_sb[:], cur[:], n_sb[:])
                nc.vector.tensor_mul(t1_sb[:], A_sb[:, :, HSL:2 * HSL],
                                     t1_sb[:])
                nc.vector.tensor_add(nxt[:], n_sb[:], t1_sb[:])
                nc.sync.dma_start(d_hy[t].rearrange("h p u -> p h u"), nxt[:])
                if t == MAXLEN:
                    break
                # exchange h slices (transposed)
                for half in range(NH):
                    pt = ps_sh.tile([128, 128], f32, tag="shp")
                    nc.tensor.transpose(pt[:HSL, :], nxt[:, half, :], ident_t[:])
                    nc.scalar.copy(hslT_send[:, half * 128:(half + 1) * 128],
                                   pt[:HSL, :])
                hx_in = dram.tile([HSL, B], f32, tag="hx_in")
                hx_out = dram.tile([NCORES, HSL, B], f32, tag="hx_out")
                nc.sync.dma_start(hx_in[:], hslT_send[:])
                nc.gpsimd.collective_compute(
                    "AllGather", OP.bypass,
                    replica_groups=[list(range(NCORES))],
                    ins=[hx_in.opt()], outs=[hx_out.opt()])
                hx_flat = hx_out[:].rearrange("c r b -> (c r) b")
                for k in range(KT):
                    rw = min(128, HPAD - k * 128)
                    nc.sync.dma_start(hT[:rw, k, :],
                                      hx_flat[k * 128:k * 128 + rw, :])
                if MODE == "f32r":
                    nc.vector.tensor_copy(hT_r[:], hT[:])

                # logits + sampling
                for half in range(NH):
                    for c in range(NCH):
                        c0 = c * CHUNK
                        cw = min(CHUNK, VSH - c0)
                        pl = ps_log.tile([128, CHUNK], f32, tag="pl")
                        for s0 in range(0, cw, 512):
                            sw = min(512, cw - s0)
                            for k in range(KT):
                                nc.tensor.matmul(
                                    pl[:, s0:s0 + sw], lhs(k, half),
                                    w_r[:, k, c0 + s0:c0 + s0 + sw],
                                    start=(k == 0), stop=(k == KT - 1))
                        nz = sbn.tile([128, CHUNK], f32, tag="big")
                        nc.sync.dma_start(nz[:, :cw],
                                          d_noise[t, half, :, c0:c0 + cw])
                        nc.vector.tensor_add(nz[:, :cw], pl[:, :cw], nz[:, :cw])
                        nc.vector.max(out=m8_all[:, half, c], in_=nz[:, :cw])
                        nc.vector.max_index(out=i8_all[:, half, c],
                                            in_max=m8_all[:, half, c],
                                            in_values=nz[:, :cw])
                emit_gh()
                # local combine (both halves at once)
                m_cat = m8_all[:, :, :, 0]
                nc.vector.tensor_reduce(Mloc[:].squeeze(2), m_cat,
                                        axis=AX.X, op=OP.max)
                nc.vector.tensor_copy(if32[:], i8_all[:, :, :, 0])
                nc.vector.tensor_add(if32[:], if32[:], cbase_t[:])
                nc.vector.tensor_tensor(
                    out=eq_sb[:], in0=m_cat,
                    in1=Mloc[:].to_broadcast([128, NH, NCH]), op=OP.is_equal)
                nc.vector.tensor_scalar_sub(if32[:], if32[:], BIG)
                nc.vector.tensor_mul(if32[:], eq_sb[:], if32[:])
                nc.vector.tensor_scalar_add(if32[:], if32[:], BIG)
                nc.vector.tensor_reduce(Ag[:].squeeze(2), if32[:],
                                        axis=AX.X, op=OP.min)
                nc.vector.tensor_copy(ex_in[:, 0:2], Mloc[:].squeeze(2))
                nc.vector.tensor_copy(ex_in[:, 2:4], Ag[:].squeeze(2))
                ei = dram.tile([128, 4], f32, tag="ex_i")
                eo = dram.tile([NCORES, 128, 4], f32, tag="ex_o")
                nc.sync.dma_start(ei[:], ex_in[:])
                nc.gpsimd.collective_compute(
                    "AllGather", OP.bypass,
                    replica_groups=[list(range(NCORES))],
                    ins=[ei.opt()], outs=[eo.opt()])
                nc.sync.dma_start(exl[:], eo[:].rearrange("c p v -> p c v"))
                m_v = exl[:, :, 0:2].rearrange("p c v -> p v c")
                a_v = exl[:, :, 2:4].rearrange("p c v -> p v c")
                nc.vector.tensor_reduce(MG[:].squeeze(2), m_v, axis=AX.X,
                                        op=OP.max)
                nc.vector.tensor_tensor(
                    out=eq2[:], in0=m_v,
                    in1=MG[:].to_broadcast([128, NH, NCORES]), op=OP.is_equal)
                nc.vector.tensor_scalar_sub(av_t[:], a_v, BIG)
                nc.vector.tensor_mul(av_t[:], eq2[:], av_t[:])
                nc.vector.tensor_scalar_add(av_t[:], av_t[:], BIG)
                nc.vector.tensor_reduce(Ag[:].squeeze(2), av_t[:], axis=AX.X,
                                        op=OP.min)
                nc.vector.tensor_copy(tok_i32[:].squeeze(2), Ag[:].squeeze(2))
                nc.vector.tensor_copy(y_sb[:, :, t], tok_i32[:].squeeze(2))
                if t == MAXLEN - 1:
                    nc.vector.memset(tok_i32[:], EOS_IDX)

            nc.sync.dma_start(d_y[:], y_sb[:])

    nc.compile()
    return nc


def _host_prep(inputs):
    z = np.asarray(inputs["z"], np.float32)
    l = np.asarray(inputs["l"], np.int32)
    emb_w = np.ascontiguousarray(np.asarray(inputs["emb_w"], np.float32))
    attr_w = np.asarray(inputs["attr_w"], np.float32)
    W_ih = np.asarray(inputs["W_ih"], np.float32)
    W_hh = np.asarray(inputs["W_hh"], np.float32)
    b_ih = np.asarray(inputs["b_ih"], np.float32)
    b_hh = np.asarray(inputs["b_hh"], np.float32)
    out_w = np.asarray(inputs["out_w"], np.float32)
    out_b = np.asarray(inputs["out_b"], np.float32)

    if "noise" not in _cache:
        _cache["noise"] = _gumbel_noise()
    g = _cache["noise"] + np.pad(
        out_b, (0, VPAD - VOCAB), constant_values=0.0)[None, None, :]
    g[:, :, VOCAB:] = -1e30

    h0 = np.concatenate([z, attr_w[l]], axis=1)
    h0p = np.zeros((B, HPAD), np.float32)
    h0p[:, :HID] = h0
    h0T = h0p.T
    h0t = np.zeros((KT, 128, B), np.float32)
    for k in range(KT):
        r = min(128, HPAD - k * 128)
        h0t[k, :r] = h0T[k * 128:k * 128 + r]

    wT = np.zeros((KT * 128, VPAD), np.float32)
    wT[:HID, :VOCAB] = out_w.T
    ident = np.eye(128, dtype=np.float32)

    in_maps = []
    for c in range(NCORES):
        base = c * VSH
        wkn = np.ascontiguousarray(
            wT[:, base:base + VSH].reshape(KT, 128, VSH))
        noise_c = np.ascontiguousarray(
            g[:, :, base:base + VSH].reshape(MAXLEN, NH, 128, VSH))
        u0 = c * HSL
        rows = np.array([gate * HID + u0 + u if u0 + u < HID else -1
                         for gate in range(3) for u in range(HSL)])
        ok = rows >= 0
        wih_sl = np.zeros((GSL, EMB), np.float32)
        whh_sl = np.zeros((GSL, HPAD), np.float32)
        b_ih_sl = np.zeros((GSL,), np.float32)
        b_hh_sl = np.zeros((GSL,), np.float32)
        wih_sl[ok] = W_ih[rows[ok]]
        whh_sl[ok, :HID] = W_hh[rows[ok]]
        b_ih_sl[ok] = b_ih[rows[ok]]
        b_hh_sl[ok] = b_hh[rows[ok]]
        wih_t = np.zeros((KTE, 128, GSL), np.float32)
        wihT = wih_sl.T
        for k in range(KTE):
            r = min(128, EMB - k * 128)
            wih_t[k, :r] = wihT[k * 128:k * 128 + r]
        whh_t = np.zeros((KT, 128, GSL), np.float32)
        whhT = whh_sl.T  # (704, 264)
        for k in range(KT):
            r = min(128, HPAD - k * 128)
            whh_t[k, :r] = whhT[k * 128:k * 128 + r]
        h0sl = np.zeros((128, NH, HSL), np.float32)
        for half in range(NH):
            h0sl[:, half, :] = h0p[half * 128:(half + 1) * 128, u0:u0 + HSL]
        cbase = np.zeros((128, NH, NCH), np.float32)
        for ci in range(NCH):
            cbase[:, :, ci] = base + ci * CHUNK
        in_maps.append({
            "wkn": wkn, "noise": noise_c, "embw": emb_w,
            "wih": wih_t, "whh": whh_t,
            "bih": np.broadcast_to(b_ih_sl, (128, GSL)).copy(),
            "bhh": np.broadcast_to(b_hh_sl, (128, GSL)).copy(),
            "h0t": h0t, "h0sl": h0sl, "cbase": cbase, "ident": ident,
        })
    return in_maps


def kernel(**inputs):
    from concourse import bass_utils
    if "nc" not in _cache:
        _cache["nc"] = _build_bass()
    nc = _cache["nc"]
    in_maps = _host_prep(inputs)
    res = bass_utils.run_bass_kernel_spmd(
        nc, in_maps, core_ids=list(range(NCORES)),
        trace=bool(_cache.get("trace", False)))
    _cache["last_result"] = res
    outs = res.results
    hy = np.zeros((B, MAXLEN + 1, HID), np.float32)
    for c in range(NCORES):
        hp = outs[c]["hy_part"]  # (31, 2, 128, 88)
        u0 = c * HSL
        w = min(HSL, HID - u0)
        if w <= 0:
            continue
        for half in range(NH):
            hy[half * 128:(half + 1) * 128, :, u0:u0 + w] = \
                hp[:, half, :, :w].transpose(1, 0, 2)
    yo = outs[0]["y_out"]
    y = np.zeros((B, MAXLEN + 1), np.int32)
    for half in range(NH):
        y[half * 128:(half + 1) * 128, :MAXLEN] = yo[:, half, :MAXLEN]
    y[:, MAXLEN] = EOS_IDX
    return hy, y
